# revision 30
# baseline (speedup 1.0000x reference)
"""Trainium2 Bass kernel for nn_Eq2to2 (Maron et al. equivariant 2->2 layer).

Math (per batch n, with x[n,d,i,j] = inputs[n,i,j,d], W_b = coefs[:,:,b]):
  out[n,i,j,s] = LeakyReLU( sum_d W9[d,s] x[n,d,i,j] + W10[d,s] x[n,d,j,i]
                 + U[n,j,s] + V[n,i,s] + G[n,s] + bias[s]
                 + [i==j] (Dd[n,i,s] + E[n,s] + diag_bias[s]) )
  U = c@W5 + r@W6 + diag@W12, V = c@W7 + r@W8 + diag@W11
  Dd = diag@W0 + r@W2 + c@W3, G = tr@W13 + S@W14, E = tr@W1 + S@W4
  r = row sums, c = col sums, diag = diagonal, tr/S = their totals.

Sharding: 8 cores = (batch n = core//2) x (row-half = core%2), with the FULL
out_dim per core. The basis is equivariant under simultaneous row+col
permutation, so the host sends each core a rotated matrix
x'[a,b] = x[(a+off)%128, (b+off)%128] and every core runs the identical
program on rows 0..63 of its rotated view; the host un-rotates the output.

Per core (everything bf16 except PSUM accumulation; tolerance is 2e-2):
  load: x' arrives host-transposed as xt[d, a*128+b] in 8 chunks; as each
    chunk lands the PE accumulates column sums into PSUM via accumulate-copy
    matmuls (identity stationary; also keeps the PE clock-gate warm) and the
    DVE reduces row sums.
  reduced terms: U/V'/Dd projections as [s, 128] via 13 small matmuls.
  main: per quad (4 output rows a), uq = U + V' built on Pool (plus the
    diagonal correction folded into 4 strided columns), two dense N=512
    matmuls (W9|W10 stationary), one DVE add psum+uq -> bf16 tmp, 4 PE
    transposes to [b, (t,s)], LeakyReLU on ACT fused with the PSUM->SBUF
    move, one DMA per quad with contiguous 1KB runs ([b, a_local, s] layout;
    host transposes back).
"""

import os
import sys

if "/opt/trn_rl_repo" not in sys.path:
    sys.path.insert(0, "/opt/trn_rl_repo")

import numpy as np
import ml_dtypes

import concourse.bass as bass
import concourse.tile as tile
from concourse import bacc, mybir
from concourse.bass_utils import run_bass_kernel_spmd

B, M, D, S = 4, 128, 128, 128
RH = M // 2          # rows per core
NB = 15
NCORES = 8
F32 = mybir.dt.float32
BF16 = mybir.dt.float16
F8 = mybir.dt.float8e4
AF = mybir.ActivationFunctionType
NEG_SLOPE = 0.01
BF_NP = np.float16

# basis slots used by the projection matmuls, in issue order
PROJ_SLOTS = [5, 6, 12, 7, 8, 11, 0, 2, 3, 13, 14, 1, 4]
MM2SPLIT = os.environ.get("EQ2_MM2SPLIT", "1") == "1"


def _build_kernel():
    nc = bacc.Bacc(
        "TRN2", target_bir_lowering=False, debug=False, num_devices=NCORES
    )
    # x' as [d, a, b] (rotated per core on host), flattened [d, a*128+b]
    xtd = nc.dram_tensor("xt", [D, M * M], BF16, kind="ExternalInput")
    xt8d = nc.dram_tensor("xt8", [D, M * M], F8, kind="ExternalInput")
    wmm = nc.dram_tensor("wmm", [D, S], BF16, kind="ExternalInput")
    w108 = nc.dram_tensor("w108", [D, S], F8, kind="ExternalInput")
    wproj = nc.dram_tensor("wproj", [D, 13 * S], BF16, kind="ExternalInput")
    biasv = nc.dram_tensor("biasv", [S, 1], F32, kind="ExternalInput")
    dbiasv = nc.dram_tensor("dbiasv", [S, 1], F32, kind="ExternalInput")
    identd = nc.dram_tensor("identd", [M, M], BF16, kind="ExternalInput")
    # [b, a_local, s]; host transposes to [a, b, s]
    out_t = nc.dram_tensor("out", [M, RH, S], BF16, kind="ExternalOutput")

    with tile.TileContext(nc) as tc:
        _kernel_body(tc, nc, xtd, xt8d, wmm, w108, wproj, biasv, dbiasv, identd, out_t)

    nc.compile()
    return nc


def _kernel_body(tc, nc, xtd, xt8d, wmm, w108, wproj, biasv, dbiasv, identd, out_t):
    NCH, CH = 8, M // 8   # chunks, rows per chunk
    with (
        tc.tile_pool(name="const", bufs=1) as constp,
        tc.tile_pool(name="small", bufs=1) as smallp,
        tc.tile_pool(name="xt", bufs=1) as xtp,
    ):
        ident = constp.tile([M, M], BF16)
        wmm_sb = constp.tile([D, S], BF16)
        w108_sb = constp.tile([D, S], F8)
        wproj_sb = constp.tile([D, 13 * S], BF16)
        bias_sb = smallp.tile([S, 1], F32)
        dbias_sb = smallp.tile([S, 1], F32)

        def wp(k):
            return wproj_sb[:, k * S:(k + 1) * S]

        xt = xtp.tile([D, M * M], BF16)      # [d, a*128+b]
        ascr = smallp.tile([D, M], BF16)     # ACT accum scratch
        racc32 = smallp.tile([D, M], F32)
        xt8 = xtp.tile([D, M * M], F8)       # transposed copy [d, b*128+a]
        rbf = smallp.tile([D, M], BF16)      # row sums as [d, a]
        cbf = smallp.tile([D, M], BF16)      # col sums as [d, b]
        diagbf = smallp.tile([D, M], BF16)   # diagonal as [d, k]
        trs32 = smallp.tile([D, 2], F32)     # [trace | total]
        trsbf = smallp.tile([D, 2], BF16)
        ctmp = smallp.tile([D, M], F32)
        u_bf = smallp.tile([S, M], BF16)     # U as [s, b]
        vb_bf = smallp.tile([S, M], BF16)    # V + G + bias as [s, a]
        dc_bf = smallp.tile([S, M], BF16)    # Dd + E + diag_bias as [s, a]

        with (
            tc.tile_pool(name="cacc", bufs=1, space="PSUM") as caccp,
            tc.tile_pool(name="warm", bufs=1, space="PSUM") as warmp,
            tc.tile_pool(name="rtree", bufs=2) as rtree,
        ):
            # a few dummy transposes to start ramping the PE clock while
            # the first chunk DMA is in flight
            pw = warmp.tile([M, M], BF16)
            for _ in range(12):
                nc.tensor.transpose(
                    pw[:], ident[:], ident[:],
                )
            # column sums accumulate on PE: even rows into cols [0:128],
            # odd rows into cols [128:256] (accumulate-copy, I stationary)
            cps2 = caccp.tile([D, 2 * M], F32)
            for k in range(NCH):
                lo = k * CH * M
                eng = nc.sync if k % 2 == 0 else nc.scalar
                if k == 0:
                    H = CH * M // 2
                    eng.dma_start(xt[:, 0:H], xtd.ap()[:, 0:H])
                    eng.dma_start(xt[:, H:2 * H], xtd.ap()[:, H:2 * H])
                else:
                    eng.dma_start(
                        xt[:, lo:lo + CH * M], xtd.ap()[:, lo:lo + CH * M]
                    )
                if k == 0:
                    nc.scalar.dma_start(ident[:], identd.ap())
                for p in range(CH // 2):
                    kk = k * (CH // 2) + p
                    nc.tensor.matmul(
                        cps2[:], ident[:],
                        xt[:, kk * 2 * M:(kk + 1) * 2 * M],
                        start=(kk == 0), stop=(kk == M // 2 - 1),
                    )
                # row sums: most chunks on DVE (sub-chunked for tighter
                # pipelining); chunk 1 on a Pool add-tree to offload DVE
                a3 = xt[:, lo:lo + CH * M].rearrange("d (a b) -> d a b", a=CH)
                def pool_tree(lo_a, n):
                    t3 = rtree.tile([D, n * 64], BF16)
                    v = t3[:].rearrange("d (a b) -> d a b", a=n)
                    src_ = a3[:, lo_a:lo_a + n, :]
                    nc.gpsimd.tensor_add(
                        v, src_[:, :, 0:64], src_[:, :, 64:128]
                    )
                    w = 32
                    while w >= 1:
                        nc.gpsimd.tensor_add(
                            v[:, :, 0:w], v[:, :, 0:w], v[:, :, w:2 * w]
                        )
                        w //= 2
                    with nc.allow_low_precision(reason="fp16 row sums ok"):
                        nc.gpsimd.tensor_copy(
                            rbf[:, k * CH + lo_a:k * CH + lo_a + n],
                            v[:, :, 0:1].squeeze(2),
                        )

                def dve_red(lo_a, n):
                    with nc.allow_low_precision(reason="fp16 row sums ok"):
                        nc.vector.reduce_sum(
                            rbf[:, k * CH + lo_a:k * CH + lo_a + n],
                            a3[:, lo_a:lo_a + n, :],
                            axis=mybir.AxisListType.X,
                        )

                if k in (1, 3):
                    pool_tree(0, CH)
                elif k == 5:
                    pool_tree(0, 8)
                    dve_red(8, 8)
                else:
                    dve_red(0, 8)
                    dve_red(8, 8)

            # second-phase loads: fp8 transposed copy + weights
            H8 = M * M // 4
            for k8 in range(4):
                eng = nc.sync if k8 % 2 == 0 else nc.scalar
                eng.dma_start(
                    xt8[:, k8 * H8:(k8 + 1) * H8],
                    xt8d.ap()[:, k8 * H8:(k8 + 1) * H8],
                )
            nc.sync.dma_start(wmm_sb[:], wmm.ap())
            nc.scalar.dma_start(w108_sb[:], w108.ap())
            nc.sync.dma_start(wproj_sb[:], wproj.ap())
            nc.sync.dma_start(bias_sb[:], biasv.ap())
            nc.sync.dma_start(dbias_sb[:], dbiasv.ap())

            # touch Lrelu once so the activation-table load happens now,
            # off the critical path (it costs ~1.3us on first use)
            nc.scalar.activation(ascr[:, 0:8], ascr[:, 8:16], AF.Lrelu,
                                 alpha=NEG_SLOPE)

            # diagonal + totals on Pool, casts on DVE/ACT
            nc.gpsimd.tensor_copy(diagbf[:], xt[:, 0:M * M:M + 1])
            nc.vector.reduce_sum(
                trs32[:, 0:1], diagbf[:], axis=mybir.AxisListType.X
            )
            nc.vector.reduce_sum(
                trs32[:, 1:2], rbf[:], axis=mybir.AxisListType.X
            )
            nc.gpsimd.tensor_copy(trsbf[:], trs32[:])
            # fold the even/odd column-sum halves
            nc.scalar.activation(ctmp[:], cps2[:, M:2 * M], AF.Identity)
            nc.vector.tensor_add(cbf[:], cps2[:, 0:M], ctmp[:])

        # ---- projections + main loop (PSUM pools coexist) ----
        u4 = u_bf[:].unsqueeze(1).broadcast_to([S, 4, M])
        with (
            tc.tile_pool(name="proj", bufs=1, space="PSUM") as projp,
            tc.tile_pool(name="p1", bufs=3, space="PSUM") as p1pool,
            tc.tile_pool(name="p2", bufs=3, space="PSUM") as p2pool,
            tc.tile_pool(name="tmp", bufs=4) as tmppool,
            tc.tile_pool(name="osb", bufs=3) as opool,
        ):
            NQ = RH // 4
            p1s = [None] * NQ
            p2s = [None] * NQ
            tmps = [None] * NQ

            def pe_mains(q):
                gi0 = 4 * q
                p1 = p1pool.tile([S, 4 * M], F32)
                nc.tensor.matmul(
                    p1[:], wmm_sb[:], xt[:, gi0 * M:(gi0 + 4) * M],
                    start=True, stop=False,
                )
                # odd quads add U on Pool instead of a preU matmul, to
                # rebalance PE vs the idle Pool engine in the drain
                nc.tensor.matmul(
                    p1[:], w108_sb[:], xt8[:, gi0 * M:(gi0 + 4) * M],
                    start=False, stop=(q % 2 == 1),
                )
                p1s[q] = p1

            def preu(q):
                # U added into PSUM last (identity stationary, broadcast
                # moving); must follow the u_bf write
                nc.tensor.matmul(
                    p1s[q][:].rearrange("s (t b) -> s t b", t=4),
                    ident[:], u4, start=False, stop=True,
                )

            def uv_mains(q):
                gi0 = 4 * q
                vbq = vb_bf[:, gi0:gi0 + 4].unsqueeze(2).broadcast_to(
                    [S, 4, M]
                )
                tmp = tmppool.tile([S, 4 * M], BF16)
                t3 = tmp[:].rearrange("s (t b) -> s t b", t=4)
                nc.vector.tensor_add(
                    t3, p1s[q][:].rearrange("s (t b) -> s t b", t=4), vbq,
                )
                if q % 2 == 1:
                    nc.gpsimd.tensor_add(t3, t3, u4)
                # diagonal correction: columns t*128 + (gi0+t)
                dv = tmp[:, gi0:gi0 + 3 * (M + 1) + 1:M + 1]
                nc.gpsimd.tensor_add(dv, dv, dc_bf[:, gi0:gi0 + 4])
                tmps[q] = tmp

            def backend(q):
                p2 = p2pool.tile([M, 4 * S], BF16)
                tmp = tmps[q]
                for t in range(4):
                    nc.tensor.transpose(
                        p2[:, t * S:(t + 1) * S],
                        tmp[:, t * M:(t + 1) * M],
                        ident[:],
                    )
                osb = opool.tile([M, 4 * S], BF16)
                nc.scalar.activation(
                    osb[:], p2[:], AF.Lrelu, alpha=NEG_SLOPE
                )
                dst = out_t.ap()[:, 4 * q:4 * q + 4, :]
                nc.sync.dma_start(
                    dst, osb[:].rearrange("b (t s) -> b t s", t=4)
                )

            pe_mains(0)
            pe_mains(1)
            pe_mains(2)

            # projections of the reduced quantities
            puT = projp.tile([S, M], F32)
            pu = puT[:]
            pallB = projp.tile([S, 2 * M + 2], F32)
            pv = pallB[:, 0:M]
            pdd = pallB[:, M:2 * M]
            pge = pallB[:, 2 * M:2 * M + 2]
            nc.tensor.matmul(pu, wp(0), cbf[:], start=True, stop=False)
            nc.tensor.matmul(pu, wp(1), rbf[:], start=False, stop=False)
            nc.tensor.matmul(pu, wp(2), diagbf[:], start=False, stop=True)
            nc.tensor.matmul(pv, wp(3), cbf[:], start=True, stop=False)
            nc.tensor.matmul(pv, wp(4), rbf[:], start=False, stop=False)
            nc.tensor.matmul(pv, wp(5), diagbf[:], start=False, stop=True)
            nc.tensor.matmul(
                pge[:, 0:1], wp(9), trsbf[:, 0:1], start=True, stop=False)
            nc.tensor.matmul(
                pge[:, 0:1], wp(10), trsbf[:, 1:2], start=False, stop=True)
            nc.tensor.matmul(
                pge[:, 1:2], wp(11), trsbf[:, 0:1], start=True, stop=False)
            nc.tensor.matmul(
                pge[:, 1:2], wp(12), trsbf[:, 1:2], start=False, stop=True)
            nc.tensor.matmul(pdd, wp(6), diagbf[:], start=True, stop=False)
            nc.tensor.matmul(pdd, wp(7), rbf[:], start=False, stop=False)
            nc.tensor.matmul(pdd, wp(8), cbf[:], start=False, stop=True)

            nc.scalar.activation(u_bf[:], pu, AF.Identity)
            nc.vector.tensor_scalar(
                vb_bf[:], pv, pge[:, 0:1], bias_sb[:, 0:1],
                op0=mybir.AluOpType.add, op1=mybir.AluOpType.add,
            )
            nc.vector.tensor_scalar(
                dc_bf[:], pdd, pge[:, 1:2], dbias_sb[:, 0:1],
                op0=mybir.AluOpType.add, op1=mybir.AluOpType.add,
            )

            # software-pipelined main loop: backend lags mains by 2
            preu(0)
            preu(2)
            uv_mains(0)
            uv_mains(1)
            uv_mains(2)
            backend(0)
            for q in range(3, NQ):
                pe_mains(q)
                if q % 2 == 0:
                    preu(q)
                uv_mains(q)
                backend(q - 2)
            backend(NQ - 2)
            backend(NQ - 1)


_CACHE = {}


def _get_nc():
    if "nc" not in _CACHE:
        _CACHE["nc"] = _build_kernel()
    return _CACHE["nc"]


def make_in_maps(inputs, coefs, bias, diag_bias):
    import ml_dtypes as _mld
    F8_NP = _mld.float8_e4m3
    eye = np.eye(M, dtype=np.float32).astype(BF_NP)
    wmm_np = np.ascontiguousarray(coefs[:, :, 9]).astype(BF_NP)
    w108_np = np.ascontiguousarray(coefs[:, :, 10]).astype(F8_NP)
    wproj_np = np.ascontiguousarray(
        np.concatenate([coefs[:, :, b] for b in PROJ_SLOTS], axis=1)
    ).astype(BF_NP)
    bias_np = np.ascontiguousarray(bias.reshape(S, 1))
    dbias_np = np.ascontiguousarray(diag_bias.reshape(S, 1))
    in_maps = []
    for core in range(NCORES):
        n, h = core // 2, core % 2
        off = h * RH
        xd = inputs[n].transpose(2, 0, 1)  # [d, i, j]
        if off:
            xd = np.roll(np.roll(xd, -off, axis=1), -off, axis=2)
        in_maps.append({
            "xt": np.ascontiguousarray(xd.reshape(D, M * M)).astype(BF_NP),
            "xt8": np.ascontiguousarray(
                xd.transpose(0, 2, 1).reshape(D, M * M)
            ).astype(F8_NP),
            "wmm": wmm_np,
            "w108": w108_np,
            "wproj": wproj_np,
            "biasv": bias_np,
            "dbiasv": dbias_np,
            "identd": eye,
        })
    return in_maps


def kernel(inputs, coefs, bias, diag_bias):
    inputs = np.ascontiguousarray(np.asarray(inputs, dtype=np.float32))
    coefs = np.asarray(coefs, dtype=np.float32)
    bias = np.asarray(bias, dtype=np.float32).reshape(-1)
    diag_bias = np.asarray(diag_bias, dtype=np.float32).reshape(-1)

    nc = _get_nc()
    in_maps = make_in_maps(inputs, coefs, bias, diag_bias)
    # the runtime occasionally reports a transient device-unrecoverable
    # state left over from a previous process; a retry clears it
    last_exc = None
    for attempt in range(3):
        try:
            res = run_bass_kernel_spmd(
                nc, in_maps, core_ids=list(range(NCORES))
            )
            break
        except Exception as e:  # noqa: BLE001
            last_exc = e
            import time as _time
            _time.sleep(10 * (attempt + 1))
    else:
        raise last_exc

    out = np.empty((B, M, M, S), dtype=np.float32)
    for core in range(NCORES):
        n, h = core // 2, core % 2
        off = h * RH
        # [b, a, s] -> [a, b, s], then undo the column rotation
        blk = res.results[core]["out"].astype(np.float32).transpose(1, 0, 2)
        if off:
            blk = np.roll(blk, off, axis=1)
        out[n, off:off + RH, :, :] = blk
    return out


# revision 31
# speedup vs baseline: 1.0214x; 1.0214x over previous
"""Trainium2 Bass kernel for nn_Eq2to2 (Maron et al. equivariant 2->2 layer).

Math (per batch n, with x[n,d,i,j] = inputs[n,i,j,d], W_b = coefs[:,:,b]):
  out[n,i,j,s] = LeakyReLU( sum_d W9[d,s] x[n,d,i,j] + W10[d,s] x[n,d,j,i]
                 + U[n,j,s] + V[n,i,s] + G[n,s] + bias[s]
                 + [i==j] (Dd[n,i,s] + E[n,s] + diag_bias[s]) )
  U = c@W5 + r@W6 + diag@W12, V = c@W7 + r@W8 + diag@W11
  Dd = diag@W0 + r@W2 + c@W3, G = tr@W13 + S@W14, E = tr@W1 + S@W4
  r = row sums, c = col sums, diag = diagonal, tr/S = their totals.

Sharding: 8 cores = (batch n = core//2) x (row-half = core%2), with the FULL
out_dim per core. The basis is equivariant under simultaneous row+col
permutation, so the host sends each core a rotated matrix
x'[a,b] = x[(a+off)%128, (b+off)%128] and every core runs the identical
program on rows 0..63 of its rotated view; the host un-rotates the output.

Per core (everything bf16 except PSUM accumulation; tolerance is 2e-2):
  load: x' arrives host-transposed as xt[d, a*128+b] in 8 chunks; as each
    chunk lands the PE accumulates column sums into PSUM via accumulate-copy
    matmuls (identity stationary; also keeps the PE clock-gate warm) and the
    DVE reduces row sums.
  reduced terms: U/V'/Dd projections as [s, 128] via 13 small matmuls.
  main: per quad (4 output rows a), uq = U + V' built on Pool (plus the
    diagonal correction folded into 4 strided columns), two dense N=512
    matmuls (W9|W10 stationary), one DVE add psum+uq -> bf16 tmp, 4 PE
    transposes to [b, (t,s)], LeakyReLU on ACT fused with the PSUM->SBUF
    move, one DMA per quad with contiguous 1KB runs ([b, a_local, s] layout;
    host transposes back).
"""

import os
import sys

if "/opt/trn_rl_repo" not in sys.path:
    sys.path.insert(0, "/opt/trn_rl_repo")

import numpy as np
import ml_dtypes

import concourse.bass as bass
import concourse.tile as tile
from concourse import bacc, mybir
from concourse.bass_utils import run_bass_kernel_spmd

B, M, D, S = 4, 128, 128, 128
RH = M // 2          # rows per core
NB = 15
NCORES = 8
F32 = mybir.dt.float32
BF16 = mybir.dt.float16
F8 = mybir.dt.float8e4
AF = mybir.ActivationFunctionType
NEG_SLOPE = 0.01
BF_NP = np.float16

# basis slots used by the projection matmuls, in issue order
PROJ_SLOTS = [5, 6, 12, 7, 8, 11, 0, 2, 3, 13, 14, 1, 4]
MM2SPLIT = os.environ.get("EQ2_MM2SPLIT", "1") == "1"


def _build_kernel():
    nc = bacc.Bacc(
        "TRN2", target_bir_lowering=False, debug=False, num_devices=NCORES
    )
    # x' as [d, a, b] (rotated per core on host), flattened [d, a*128+b]
    xtd = nc.dram_tensor("xt", [D, M * M], BF16, kind="ExternalInput")
    xt8d = nc.dram_tensor("xt8", [D, M * M], F8, kind="ExternalInput")
    wmm = nc.dram_tensor("wmm", [D, S], BF16, kind="ExternalInput")
    w108 = nc.dram_tensor("w108", [D, S], F8, kind="ExternalInput")
    wproj = nc.dram_tensor("wproj", [D, 13 * S], BF16, kind="ExternalInput")
    biasv = nc.dram_tensor("biasv", [S, 1], F32, kind="ExternalInput")
    dbiasv = nc.dram_tensor("dbiasv", [S, 1], F32, kind="ExternalInput")
    identd = nc.dram_tensor("identd", [M, M], BF16, kind="ExternalInput")
    # [b, a_local, s]; host transposes to [a, b, s]
    out_t = nc.dram_tensor("out", [M, RH, S], BF16, kind="ExternalOutput")

    with tile.TileContext(nc) as tc:
        _kernel_body(tc, nc, xtd, xt8d, wmm, w108, wproj, biasv, dbiasv, identd, out_t)

    nc.compile()
    return nc


def _kernel_body(tc, nc, xtd, xt8d, wmm, w108, wproj, biasv, dbiasv, identd, out_t):
    NCH, CH = 8, M // 8   # chunks, rows per chunk
    with (
        tc.tile_pool(name="const", bufs=1) as constp,
        tc.tile_pool(name="small", bufs=1) as smallp,
        tc.tile_pool(name="xt", bufs=1) as xtp,
    ):
        ident = constp.tile([M, M], BF16)
        wmm_sb = constp.tile([D, S], BF16)
        w108_sb = constp.tile([D, S], F8)
        wproj_sb = constp.tile([D, 13 * S], BF16)
        bias_sb = smallp.tile([S, 1], F32)
        dbias_sb = smallp.tile([S, 1], F32)

        def wp(k):
            return wproj_sb[:, k * S:(k + 1) * S]

        xt = xtp.tile([D, M * M], BF16)      # [d, a*128+b]
        ascr = smallp.tile([D, M], BF16)     # ACT accum scratch
        racc32 = smallp.tile([D, M], F32)
        xt8 = xtp.tile([D, M * M], F8)       # transposed copy [d, b*128+a]
        rbf = smallp.tile([D, M], BF16)      # row sums as [d, a]
        cbf = smallp.tile([D, M], BF16)      # col sums as [d, b]
        diagbf = smallp.tile([D, M], BF16)   # diagonal as [d, k]
        trs32 = smallp.tile([D, 2], F32)     # [trace | total]
        trsbf = smallp.tile([D, 2], BF16)
        ctmp = smallp.tile([D, M], F32)
        u_bf = smallp.tile([S, M], BF16)     # U as [s, b]
        vb_bf = smallp.tile([S, M], BF16)    # V + G + bias as [s, a]
        dc_bf = smallp.tile([S, M], BF16)    # Dd + E + diag_bias as [s, a]

        with (
            tc.tile_pool(name="cacc", bufs=1, space="PSUM") as caccp,
            tc.tile_pool(name="warm", bufs=1, space="PSUM") as warmp,
            tc.tile_pool(name="rtree", bufs=2) as rtree,
        ):
            # a few dummy transposes to start ramping the PE clock while
            # the first chunk DMA is in flight
            pw = warmp.tile([M, M], BF16)
            for _ in range(12):
                nc.tensor.transpose(
                    pw[:], ident[:], ident[:],
                )
            # column sums accumulate on PE: even rows into cols [0:128],
            # odd rows into cols [128:256] (accumulate-copy, I stationary)
            cps2 = caccp.tile([D, 2 * M], F32)
            for k in range(NCH):
                lo = k * CH * M
                eng = nc.sync if k % 2 == 0 else nc.scalar
                if k == 0:
                    H = CH * M // 2
                    eng.dma_start(xt[:, 0:H], xtd.ap()[:, 0:H])
                    eng.dma_start(xt[:, H:2 * H], xtd.ap()[:, H:2 * H])
                else:
                    eng.dma_start(
                        xt[:, lo:lo + CH * M], xtd.ap()[:, lo:lo + CH * M]
                    )
                if k == 0:
                    nc.scalar.dma_start(ident[:], identd.ap())
                for p in range(CH // 2):
                    kk = k * (CH // 2) + p
                    nc.tensor.matmul(
                        cps2[:], ident[:],
                        xt[:, kk * 2 * M:(kk + 1) * 2 * M],
                        start=(kk == 0), stop=(kk == M // 2 - 1),
                    )
                # row sums: most chunks on DVE (sub-chunked for tighter
                # pipelining); chunk 1 on a Pool add-tree to offload DVE
                a3 = xt[:, lo:lo + CH * M].rearrange("d (a b) -> d a b", a=CH)
                def pool_tree(lo_a, n):
                    t3 = rtree.tile([D, n * 64], BF16)
                    v = t3[:].rearrange("d (a b) -> d a b", a=n)
                    src_ = a3[:, lo_a:lo_a + n, :]
                    nc.gpsimd.tensor_add(
                        v, src_[:, :, 0:64], src_[:, :, 64:128]
                    )
                    w = 32
                    while w >= 1:
                        nc.gpsimd.tensor_add(
                            v[:, :, 0:w], v[:, :, 0:w], v[:, :, w:2 * w]
                        )
                        w //= 2
                    with nc.allow_low_precision(reason="fp16 row sums ok"):
                        nc.gpsimd.tensor_copy(
                            rbf[:, k * CH + lo_a:k * CH + lo_a + n],
                            v[:, :, 0:1].squeeze(2),
                        )

                def dve_red(lo_a, n):
                    with nc.allow_low_precision(reason="fp16 row sums ok"):
                        nc.vector.reduce_sum(
                            rbf[:, k * CH + lo_a:k * CH + lo_a + n],
                            a3[:, lo_a:lo_a + n, :],
                            axis=mybir.AxisListType.X,
                        )

                if k in (1, 3):
                    pool_tree(0, CH)
                elif k == 5:
                    pool_tree(0, 8)
                    dve_red(8, 8)
                else:
                    dve_red(0, 8)
                    dve_red(8, 8)

            # second-phase loads: fp8 transposed copy + weights
            H8 = M * M // 4
            for k8 in range(4):
                eng = nc.sync if k8 % 2 == 0 else nc.scalar
                eng.dma_start(
                    xt8[:, k8 * H8:(k8 + 1) * H8],
                    xt8d.ap()[:, k8 * H8:(k8 + 1) * H8],
                )
            nc.sync.dma_start(wmm_sb[:], wmm.ap())
            nc.scalar.dma_start(w108_sb[:], w108.ap())
            nc.sync.dma_start(wproj_sb[:], wproj.ap())
            nc.sync.dma_start(bias_sb[:], biasv.ap())
            nc.sync.dma_start(dbias_sb[:], dbiasv.ap())

            # touch Lrelu once so the activation-table load happens now,
            # off the critical path (it costs ~1.3us on first use)
            nc.scalar.activation(ascr[:, 0:8], ascr[:, 8:16], AF.Lrelu,
                                 alpha=NEG_SLOPE)

            # diagonal + totals on Pool, casts on DVE/ACT
            nc.gpsimd.tensor_copy(diagbf[:], xt[:, 0:M * M:M + 1])
            nc.vector.reduce_sum(
                trs32[:, 0:1], diagbf[:], axis=mybir.AxisListType.X
            )
            nc.vector.reduce_sum(
                trs32[:, 1:2], rbf[:], axis=mybir.AxisListType.X
            )
            nc.gpsimd.tensor_copy(trsbf[:], trs32[:])
            # fold the even/odd column-sum halves
            nc.scalar.activation(ctmp[:], cps2[:, M:2 * M], AF.Identity)
            nc.vector.tensor_add(cbf[:], cps2[:, 0:M], ctmp[:])

        # ---- projections + main loop (PSUM pools coexist) ----
        u4 = u_bf[:].unsqueeze(1).broadcast_to([S, 4, M])
        with (
            tc.tile_pool(name="proj", bufs=1, space="PSUM") as projp,
            tc.tile_pool(name="p1", bufs=3, space="PSUM") as p1pool,
            tc.tile_pool(name="p2", bufs=3, space="PSUM") as p2pool,
            tc.tile_pool(name="tmp", bufs=4) as tmppool,
            tc.tile_pool(name="osb", bufs=3) as opool,
        ):
            NQ = RH // 4
            p1s = [None] * NQ
            p2s = [None] * NQ
            tmps = [None] * NQ

            def pe_mains(q):
                gi0 = 4 * q
                p1 = p1pool.tile([S, 4 * M], F32)
                nc.tensor.matmul(
                    p1[:], wmm_sb[:], xt[:, gi0 * M:(gi0 + 4) * M],
                    start=True, stop=False,
                )
                nc.tensor.matmul(
                    p1[:], w108_sb[:], xt8[:, gi0 * M:(gi0 + 4) * M],
                    start=False, stop=False,
                )
                p1s[q] = p1

            def preu(q):
                # U added into PSUM last (identity stationary, broadcast
                # moving); must follow the u_bf write
                nc.tensor.matmul(
                    p1s[q][:].rearrange("s (t b) -> s t b", t=4),
                    ident[:], u4, start=False, stop=True,
                )

            def uv_mains(q):
                gi0 = 4 * q
                vbq = vb_bf[:, gi0:gi0 + 4].unsqueeze(2).broadcast_to(
                    [S, 4, M]
                )
                tmp = tmppool.tile([S, 4 * M], BF16)
                nc.vector.tensor_add(
                    tmp[:].rearrange("s (t b) -> s t b", t=4),
                    p1s[q][:].rearrange("s (t b) -> s t b", t=4),
                    vbq,
                )
                # diagonal correction: columns t*128 + (gi0+t)
                dv = tmp[:, gi0:gi0 + 3 * (M + 1) + 1:M + 1]
                nc.gpsimd.tensor_add(dv, dv, dc_bf[:, gi0:gi0 + 4])
                tmps[q] = tmp

            def backend(q):
                p2 = p2pool.tile([M, 4 * S], BF16)
                tmp = tmps[q]
                for t in range(4):
                    nc.tensor.transpose(
                        p2[:, t * S:(t + 1) * S],
                        tmp[:, t * M:(t + 1) * M],
                        ident[:],
                    )
                osb = opool.tile([M, 4 * S], BF16)
                nc.scalar.activation(
                    osb[:], p2[:], AF.Lrelu, alpha=NEG_SLOPE
                )
                dst = out_t.ap()[:, 4 * q:4 * q + 4, :]
                nc.sync.dma_start(
                    dst, osb[:].rearrange("b (t s) -> b t s", t=4)
                )

            pe_mains(0)
            pe_mains(1)
            pe_mains(2)

            # projections of the reduced quantities
            puT = projp.tile([S, M], F32)
            pu = puT[:]
            pallB = projp.tile([S, 2 * M + 2], F32)
            pv = pallB[:, 0:M]
            pdd = pallB[:, M:2 * M]
            pge = pallB[:, 2 * M:2 * M + 2]
            nc.tensor.matmul(pu, wp(0), cbf[:], start=True, stop=False)
            nc.tensor.matmul(pu, wp(1), rbf[:], start=False, stop=False)
            nc.tensor.matmul(pu, wp(2), diagbf[:], start=False, stop=True)
            nc.tensor.matmul(pv, wp(3), cbf[:], start=True, stop=False)
            nc.tensor.matmul(pv, wp(4), rbf[:], start=False, stop=False)
            nc.tensor.matmul(pv, wp(5), diagbf[:], start=False, stop=True)
            nc.tensor.matmul(
                pge[:, 0:1], wp(9), trsbf[:, 0:1], start=True, stop=False)
            nc.tensor.matmul(
                pge[:, 0:1], wp(10), trsbf[:, 1:2], start=False, stop=True)
            nc.tensor.matmul(
                pge[:, 1:2], wp(11), trsbf[:, 0:1], start=True, stop=False)
            nc.tensor.matmul(
                pge[:, 1:2], wp(12), trsbf[:, 1:2], start=False, stop=True)
            nc.tensor.matmul(pdd, wp(6), diagbf[:], start=True, stop=False)
            nc.tensor.matmul(pdd, wp(7), rbf[:], start=False, stop=False)
            nc.tensor.matmul(pdd, wp(8), cbf[:], start=False, stop=True)

            nc.scalar.activation(u_bf[:], pu, AF.Identity)
            nc.vector.tensor_scalar(
                vb_bf[:], pv, pge[:, 0:1], bias_sb[:, 0:1],
                op0=mybir.AluOpType.add, op1=mybir.AluOpType.add,
            )
            nc.vector.tensor_scalar(
                dc_bf[:], pdd, pge[:, 1:2], dbias_sb[:, 0:1],
                op0=mybir.AluOpType.add, op1=mybir.AluOpType.add,
            )

            # software-pipelined main loop: backend lags mains by 2
            preu(0)
            preu(1)
            preu(2)
            uv_mains(0)
            uv_mains(1)
            uv_mains(2)
            backend(0)
            for q in range(3, NQ):
                pe_mains(q)
                preu(q)
                uv_mains(q)
                backend(q - 2)
            backend(NQ - 2)
            backend(NQ - 1)


_CACHE = {}


def _get_nc():
    if "nc" not in _CACHE:
        _CACHE["nc"] = _build_kernel()
    return _CACHE["nc"]


def make_in_maps(inputs, coefs, bias, diag_bias):
    import ml_dtypes as _mld
    F8_NP = _mld.float8_e4m3
    eye = np.eye(M, dtype=np.float32).astype(BF_NP)
    wmm_np = np.ascontiguousarray(coefs[:, :, 9]).astype(BF_NP)
    w108_np = np.ascontiguousarray(coefs[:, :, 10]).astype(F8_NP)
    wproj_np = np.ascontiguousarray(
        np.concatenate([coefs[:, :, b] for b in PROJ_SLOTS], axis=1)
    ).astype(BF_NP)
    bias_np = np.ascontiguousarray(bias.reshape(S, 1))
    dbias_np = np.ascontiguousarray(diag_bias.reshape(S, 1))
    in_maps = []
    for core in range(NCORES):
        n, h = core // 2, core % 2
        off = h * RH
        xd = inputs[n].transpose(2, 0, 1)  # [d, i, j]
        if off:
            xd = np.roll(np.roll(xd, -off, axis=1), -off, axis=2)
        in_maps.append({
            "xt": np.ascontiguousarray(xd.reshape(D, M * M)).astype(BF_NP),
            "xt8": np.ascontiguousarray(
                xd.transpose(0, 2, 1).reshape(D, M * M)
            ).astype(F8_NP),
            "wmm": wmm_np,
            "w108": w108_np,
            "wproj": wproj_np,
            "biasv": bias_np,
            "dbiasv": dbias_np,
            "identd": eye,
        })
    return in_maps


def kernel(inputs, coefs, bias, diag_bias):
    inputs = np.ascontiguousarray(np.asarray(inputs, dtype=np.float32))
    coefs = np.asarray(coefs, dtype=np.float32)
    bias = np.asarray(bias, dtype=np.float32).reshape(-1)
    diag_bias = np.asarray(diag_bias, dtype=np.float32).reshape(-1)

    nc = _get_nc()
    in_maps = make_in_maps(inputs, coefs, bias, diag_bias)
    # the runtime occasionally reports a transient device-unrecoverable
    # state left over from a previous process; a retry clears it
    last_exc = None
    for attempt in range(3):
        try:
            res = run_bass_kernel_spmd(
                nc, in_maps, core_ids=list(range(NCORES))
            )
            break
        except Exception as e:  # noqa: BLE001
            last_exc = e
            import time as _time
            _time.sleep(10 * (attempt + 1))
    else:
        raise last_exc

    out = np.empty((B, M, M, S), dtype=np.float32)
    for core in range(NCORES):
        n, h = core // 2, core % 2
        off = h * RH
        # [b, a, s] -> [a, b, s], then undo the column rotation
        blk = res.results[core]["out"].astype(np.float32).transpose(1, 0, 2)
        if off:
            blk = np.roll(blk, off, axis=1)
        out[n, off:off + RH, :, :] = blk
    return out
